# revision 9
# baseline (speedup 1.0000x reference)
"""D4-pool Trainium2 kernel.

x: [256, 128, 64, 64] f32. Groups of 8 consecutive batch entries hold the 8
D4 orientations of one image; undo each orientation and mean over the group,
giving [32, 128, 64, 64].

Sharding: data-parallel over the group dim — core k gets groups [4k, 4k+4)
(batch entries [32k, 32k+32)), so the reduce is fully device-local.

Layout: C (=128) on SBUF partitions, (H, W) on the free dim; every D4
inverse transform is free-dim address arithmetic. Per partition:
  o=0: A[h, w]          o=1: A[w, 63-h]     o=2: A[63-h, 63-w]
  o=3: A[63-w, h]       o=4: A[h, 63-w]     o=5: A[w, h]
  o=6: A[63-h, w]       o=7: A[63-w, 63-h]
Two accumulators so only one DVE op per group pays the inner-stride-64
(transposed) read: acc[c,h,w] (o=0 init, += o=2,6,4) and accT[c,w,h]
(o=5 init, += o=1,3,7, folded into acc by a transposed tensor_add).

The kernel is HBM-line-rate bound (~75.5 MB/core through 16 SDMA engines ≈
177 us), so the schedule attacks the serial tail after the last load. Each
group splits into phase A (o=0,5 inits, o=1,3,7 accT STTs, transposed
combine) and phase B (o=2,6 STTs, then o=4 in H-halves, each half stored as
soon as its STT lands — o=4 rows map straight through to output rows).
Phases interleave A0,A1,B0,A2,B1,A3,B2,B3: every B phase's DVE work (13 us)
fits its own load window, and A_{g+1}'s combine overlaps B_g's loads, so
DVE enters the final B caught up. B3's three tiles are loaded as H-halves
so each half STT chases its own load; the whole-kernel tail is one half
STT + one half store (~4 us) instead of the baseline's ~16 us
STT+combine+store chain.
"""

import sys

for _p in ("/opt/trn_rl_repo",):
    if _p not in sys.path:
        sys.path.insert(0, _p)

import numpy as np

import concourse.bacc as bacc
import concourse.mybir as mybir
from concourse.bass_utils import run_bass_kernel_spmd
from concourse.tile import TileContext

N_CORES = 8
B, C, H, W = 256, 128, 64, 64
ENTRIES_PER_CORE = B // N_CORES          # 32 batch entries
GROUPS_PER_CORE = ENTRIES_PER_CORE // 8  # 4 groups of 8 orientations


def build_nc(groups: int = GROUPS_PER_CORE) -> bacc.Bacc:
    f32 = mybir.dt.float32
    nc = bacc.Bacc()
    x = nc.declare_dram_parameter("x", [groups * 8, C, H, W], f32, isOutput=False)
    y = nc.declare_dram_parameter("y", [groups, C, H, W], f32, isOutput=True)

    # accT-side APs: accT[w,h] += A1[w,63-h] / A3[63-w,h] / A7[63-w,63-h].
    accT_slice = {1: lambda t: t[:, :, ::-1], 3: lambda t: t[:, ::-1, :],
                  7: lambda t: t[:, ::-1, ::-1]}
    # acc-side APs (input-row chunk [r0,r0+n) feeds output rows [h0,h0+n)):
    #   o=2: h0 = H-n-r0, chunk[::-1, ::-1]    o=6: h0 = H-n-r0, chunk[::-1, :]
    #   o=4: h0 = r0,     chunk[:, ::-1]
    acc_slice = {2: lambda t: t[:, ::-1, ::-1], 4: lambda t: t[:, :, ::-1],
                 6: lambda t: t[:, ::-1, :]}
    mult, add = mybir.AluOpType.mult, mybir.AluOpType.add
    HH = H // 2
    with TileContext(nc) as tc:
        with (
            tc.tile_pool(name="xin", bufs=8) as xin_pool,
            tc.tile_pool(name="acc", bufs=2) as acc_pool,
            tc.tile_pool(name="accT", bufs=2) as accT_pool,
        ):
            accs = {}

            def phase_a(g):
                acc = acc_pool.tile([C, H, W], f32, tag="acc")
                accT = accT_pool.tile([C, H, W], f32, tag="accT")
                accs[g] = acc
                for o in (0, 5, 1, 3, 7):
                    xt = xin_pool.tile([C, H, W], f32, tag="xin")
                    nc.sync.dma_start(xt[:, :, :], x[8 * g + o])
                    if o == 0:
                        nc.scalar.mul(acc[:, :, :], xt[:, :, :], 0.125)
                    elif o == 5:
                        nc.scalar.mul(accT[:, :, :], xt[:, :, :], 0.125)
                    else:
                        nc.vector.scalar_tensor_tensor(
                            accT[:, :, :], accT_slice[o](xt), 0.125,
                            accT[:, :, :], mult, add,
                        )
                # accT complete: fold into acc (accT frees here). Overlaps
                # the next emitted phase's loads.
                for h0 in (0, HH):
                    hs = slice(h0, h0 + HH)
                    nc.vector.tensor_add(
                        acc[:, hs, :], acc[:, hs, :],
                        accT[:, :, hs].transpose([0, 2, 1]),
                    )

            def phase_b(g, stream):
                acc = accs.pop(g)
                for o in (2, 6, 4):
                    xt = xin_pool.tile([C, H, W], f32, tag="xin")
                    if not stream:
                        nc.sync.dma_start(xt[:, :, :], x[8 * g + o])
                    for r0 in (0, HH):
                        rs = slice(r0, r0 + HH)
                        if stream:
                            # Half-tile load so the STT chases the DMA.
                            nc.sync.dma_start(
                                xt[:, rs, :], x[8 * g + o][:, rs, :]
                            )
                        h0 = r0 if o == 4 else H - HH - r0
                        hs = slice(h0, h0 + HH)
                        nc.vector.scalar_tensor_tensor(
                            acc[:, hs, :], acc_slice[o](xt[:, rs, :]), 0.125,
                            acc[:, hs, :], mult, add,
                        )
                        if o == 4:
                            # Rows hs complete (o=2/6 precede on DVE; the
                            # combine ran in phase A). Store on the ACT
                            # HWDGE queue to keep compute-gated stores from
                            # head-of-line blocking loads on sync's queue.
                            nc.scalar.dma_start(y[g][:, hs, :], acc[:, hs, :])

            phase_a(0)
            phase_a(1)
            phase_b(0, stream=False)
            phase_a(2)
            phase_b(1, stream=False)
            phase_a(3)
            phase_b(2, stream=False)
            phase_b(3, stream=True)
    nc.compile()
    return nc


_NC_CACHE: list = []


def run(x: np.ndarray, trace: bool = False, **spmd_kwargs):
    """Shard, run on all 8 cores, gather. Returns (output, BassKernelResults)."""
    x = np.ascontiguousarray(x, dtype=np.float32)
    assert x.shape == (B, C, H, W), x.shape
    shards = x.reshape(N_CORES, ENTRIES_PER_CORE, C, H, W)
    if not _NC_CACHE:
        _NC_CACHE.append(build_nc())
    nc = _NC_CACHE[0]
    in_maps = [{"x": shards[i]} for i in range(N_CORES)]
    res = run_bass_kernel_spmd(
        nc, in_maps, list(range(N_CORES)), trace=trace, **spmd_kwargs
    )
    out = np.concatenate([res.results[i]["y"] for i in range(N_CORES)], axis=0)
    return out, res


def kernel(x: np.ndarray) -> np.ndarray:
    out, _ = run(x)
    return out


# revision 10
# speedup vs baseline: 1.0181x; 1.0181x over previous
"""D4-pool Trainium2 kernel.

x: [256, 128, 64, 64] f32. Groups of 8 consecutive batch entries hold the 8
D4 orientations of one image; undo each orientation and mean over the group,
giving [32, 128, 64, 64].

Sharding: data-parallel over the group dim — core k gets groups [4k, 4k+4)
(batch entries [32k, 32k+32)), so the reduce is fully device-local.

Layout: C (=128) on SBUF partitions, (H, W) on the free dim; every D4
inverse transform is free-dim address arithmetic. Per partition:
  o=0: A[h, w]          o=1: A[w, 63-h]     o=2: A[63-h, 63-w]
  o=3: A[63-w, h]       o=4: A[h, 63-w]     o=5: A[w, h]
  o=6: A[63-h, w]       o=7: A[63-w, 63-h]
Two accumulators so only one DVE op per group pays the inner-stride-64
(transposed) read: acc[c,h,w] (o=0 init, += o=2,6,4) and accT[c,w,h]
(o=5 init, += o=1,3,7, folded into acc by a transposed tensor_add).

The kernel is HBM-line-rate bound (~75.5 MB/core through 16 SDMA engines ≈
177 us), so the schedule attacks the serial tail after the last load. Each
group splits into phase A (o=0,5 inits, o=1,3,7 accT STTs, transposed
combine) and phase B (o=2,6 STTs, then o=4 in H-halves, each half stored as
soon as its STT lands — o=4 rows map straight through to output rows).
Phases interleave A0,A1,B0,A2,B1,A3,B2,B3: every B phase's DVE work (13 us)
fits its own load window, and A_{g+1}'s combine overlaps B_g's loads, so
DVE enters the final B caught up. B3's three tiles are loaded as H-halves
so each half STT chases its own load; the whole-kernel tail is one half
STT + one half store (~4 us) instead of the baseline's ~16 us
STT+combine+store chain.
"""

import sys

for _p in ("/opt/trn_rl_repo",):
    if _p not in sys.path:
        sys.path.insert(0, _p)

import numpy as np

import concourse.bacc as bacc
import concourse.mybir as mybir
from concourse.bass_utils import run_bass_kernel_spmd
from concourse.tile import TileContext

N_CORES = 8
B, C, H, W = 256, 128, 64, 64
ENTRIES_PER_CORE = B // N_CORES          # 32 batch entries
GROUPS_PER_CORE = ENTRIES_PER_CORE // 8  # 4 groups of 8 orientations


def build_nc(groups: int = GROUPS_PER_CORE) -> bacc.Bacc:
    f32 = mybir.dt.float32
    nc = bacc.Bacc()
    x = nc.declare_dram_parameter("x", [groups * 8, C, H, W], f32, isOutput=False)
    y = nc.declare_dram_parameter("y", [groups, C, H, W], f32, isOutput=True)

    # accT-side APs: accT[w,h] += A1[w,63-h] / A3[63-w,h] / A7[63-w,63-h].
    accT_slice = {1: lambda t: t[:, :, ::-1], 3: lambda t: t[:, ::-1, :],
                  7: lambda t: t[:, ::-1, ::-1]}
    # acc-side APs (input-row chunk [r0,r0+n) feeds output rows [h0,h0+n)):
    #   o=2: h0 = H-n-r0, chunk[::-1, ::-1]    o=6: h0 = H-n-r0, chunk[::-1, :]
    #   o=4: h0 = r0,     chunk[:, ::-1]
    acc_slice = {2: lambda t: t[:, ::-1, ::-1], 4: lambda t: t[:, :, ::-1],
                 6: lambda t: t[:, ::-1, :]}
    mult, add = mybir.AluOpType.mult, mybir.AluOpType.add
    HH = H // 2
    with TileContext(nc) as tc:
        with (
            tc.tile_pool(name="xin", bufs=8) as xin_pool,
            tc.tile_pool(name="acc", bufs=2) as acc_pool,
            tc.tile_pool(name="accT", bufs=2) as accT_pool,
        ):
            accs = {}

            def phase_a(g):
                acc = acc_pool.tile([C, H, W], f32, tag="acc")
                accT = accT_pool.tile([C, H, W], f32, tag="accT")
                accs[g] = acc
                for o in (0, 5, 1, 3, 7):
                    xt = xin_pool.tile([C, H, W], f32, tag="xin")
                    nc.sync.dma_start(xt[:, :, :], x[8 * g + o])
                    if o == 0:
                        nc.scalar.mul(acc[:, :, :], xt[:, :, :], 0.125)
                    elif o == 5:
                        nc.scalar.mul(accT[:, :, :], xt[:, :, :], 0.125)
                    else:
                        nc.vector.scalar_tensor_tensor(
                            accT[:, :, :], accT_slice[o](xt), 0.125,
                            accT[:, :, :], mult, add,
                        )
                # accT complete: fold into acc (accT frees here). Overlaps
                # the next emitted phase's loads.
                for h0 in (0, HH):
                    hs = slice(h0, h0 + HH)
                    nc.vector.tensor_add(
                        acc[:, hs, :], acc[:, hs, :],
                        accT[:, :, hs].transpose([0, 2, 1]),
                    )

            def phase_b(g, stream):
                acc = accs.pop(g)
                for o in (2, 6, 4):
                    xt = xin_pool.tile([C, H, W], f32, tag="xin")
                    if not stream:
                        nc.sync.dma_start(xt[:, :, :], x[8 * g + o])
                    for r0 in (0, HH):
                        rs = slice(r0, r0 + HH)
                        if stream:
                            # Half-tile load so the STT chases the DMA.
                            nc.sync.dma_start(
                                xt[:, rs, :], x[8 * g + o][:, rs, :]
                            )
                        h0 = r0 if o == 4 else H - HH - r0
                        hs = slice(h0, h0 + HH)
                        nc.vector.scalar_tensor_tensor(
                            acc[:, hs, :], acc_slice[o](xt[:, rs, :]), 0.125,
                            acc[:, hs, :], mult, add,
                        )
                        if o == 4:
                            # Rows hs complete (o=2/6 precede on DVE; the
                            # combine ran in phase A). Store on the ACT
                            # HWDGE queue to keep compute-gated stores from
                            # head-of-line blocking loads on sync's queue.
                            nc.scalar.dma_start(y[g][:, hs, :], acc[:, hs, :])

            phase_a(0)
            phase_a(1)
            phase_b(0, stream=False)
            phase_a(2)
            phase_b(1, stream=False)
            phase_a(3)
            phase_b(2, stream=False)
            phase_b(3, stream=False)
    nc.compile()
    return nc


_NC_CACHE: list = []


def run(x: np.ndarray, trace: bool = False, **spmd_kwargs):
    """Shard, run on all 8 cores, gather. Returns (output, BassKernelResults)."""
    x = np.ascontiguousarray(x, dtype=np.float32)
    assert x.shape == (B, C, H, W), x.shape
    shards = x.reshape(N_CORES, ENTRIES_PER_CORE, C, H, W)
    if not _NC_CACHE:
        _NC_CACHE.append(build_nc())
    nc = _NC_CACHE[0]
    in_maps = [{"x": shards[i]} for i in range(N_CORES)]
    res = run_bass_kernel_spmd(
        nc, in_maps, list(range(N_CORES)), trace=trace, **spmd_kwargs
    )
    out = np.concatenate([res.results[i]["y"] for i in range(N_CORES)], axis=0)
    return out, res


def kernel(x: np.ndarray) -> np.ndarray:
    out, _ = run(x)
    return out


# revision 11
# speedup vs baseline: 1.1430x; 1.1227x over previous
"""D4-pool Trainium2 kernel.

x: [256, 128, 64, 64] f32. Groups of 8 consecutive batch entries hold the 8
D4 orientations of one image; undo each orientation and mean over the group,
giving [32, 128, 64, 64].

Sharding: data-parallel over the group dim — core k gets groups [4k, 4k+4)
(batch entries [32k, 32k+32)), so the reduce is fully device-local.

Layout trick: with C (=128) on SBUF partitions and (H, W) on the free dim,
every D4 inverse transform is pure free-dim address arithmetic (stride ±1 /
±64 access patterns) — no transpose instructions, no partition movement.
Per partition, the required inverse-transform reads are:
  o=0: A[h, w]          o=1: A[w, 63-h]     o=2: A[63-h, 63-w]
  o=3: A[63-w, h]       o=4: A[h, 63-w]     o=5: A[w, h]
  o=6: A[63-h, w]       o=7: A[63-w, 63-h]
Loads/stores are fully contiguous 2 MiB DMAs; DVE does the accumulation
(1/8-scale folded in); ACT initializes accumulators off the critical path.
Two accumulators per group so only ONE DVE op per group pays the slow
inner-stride-64 (transposed) read:
  acc  [c,h,w]: o=0 init (ACT), += o=2,4,6 (flip APs, stride ±1)
  accT [c,w,h]: accT-side init (ACT), += rest of o=1,3,5,7, folded into acc
                by a transposed tensor_add (the combine).

The kernel is HBM-line-rate bound (~75.5 MB/core through 16 SDMA engines at
~26.7 GiB/s ≈ 177 us), so the schedule only has two levers: keep the DMA
queue un-stalled (the sync queue issues in order, so every load's slot-free
dependency must resolve early — groups 0..2 keep the exact steady-state
emission pattern), and shorten the serial tail after the very last load.
For the last group the order is (1,5,0,3,7,2,6,4): accT is initialized from
o=1 by ACT (any orientation works — it's just a strided scaled copy), so
DVE's first op starts at the 2nd load instead of the 5th; the combine runs
mid-group overlapped by the o=2/6/4 loads; o=4 — whose rows map straight
through to output rows — is processed last in H-halves, each half stored
immediately. Tail ≈ lag + DVE work − load window ≈ 9 us vs ~16 us for the
combine-last ordering.
"""

import sys

for _p in ("/opt/trn_rl_repo",):
    if _p not in sys.path:
        sys.path.insert(0, _p)

import numpy as np

import concourse.bacc as bacc
import concourse.mybir as mybir
from concourse.bass_utils import run_bass_kernel_spmd
from concourse.tile import TileContext

N_CORES = 8
B, C, H, W = 256, 128, 64, 64
ENTRIES_PER_CORE = B // N_CORES          # 32 batch entries
GROUPS_PER_CORE = ENTRIES_PER_CORE // 8  # 4 groups of 8 orientations


def build_nc(groups: int = GROUPS_PER_CORE) -> bacc.Bacc:
    f32 = mybir.dt.float32
    nc = bacc.Bacc()
    x = nc.declare_dram_parameter("x", [groups * 8, C, H, W], f32, isOutput=False)
    y = nc.declare_dram_parameter("y", [groups, C, H, W], f32, isOutput=True)

    # accT-side views (accT[w,h] += A[...] in transposed coords):
    #   o=1: A1[w,63-h]   o=3: A3[63-w,h]   o=5: A5[w,h]   o=7: A7[63-w,63-h]
    accT_slice = {1: lambda t: t[:, :, ::-1], 3: lambda t: t[:, ::-1, :],
                  5: lambda t: t[:, :, :], 7: lambda t: t[:, ::-1, ::-1]}
    acc_slice = {2: lambda t: t[:, ::-1, ::-1], 4: lambda t: t[:, :, ::-1],
                 6: lambda t: t[:, ::-1, :]}
    mult, add = mybir.AluOpType.mult, mybir.AluOpType.add
    HH = H // 2
    with TileContext(nc) as tc:
        with (
            tc.tile_pool(name="xin", bufs=8) as xin_pool,
            tc.tile_pool(name="acc", bufs=2) as acc_pool,
            tc.tile_pool(name="accT", bufs=2) as accT_pool,
        ):
            for g in range(groups):
                acc = acc_pool.tile([C, H, W], f32, tag="acc")
                accT = accT_pool.tile([C, H, W], f32, tag="accT")
                last = g == groups - 1
                order = (1, 5, 0, 3, 7, 2, 6, 4) if last else (0, 5, 1, 2, 3, 4, 6, 7)
                acc_init, accT_init = (0, 1) if last else (0, 5)
                for o in order:
                    xt = xin_pool.tile([C, H, W], f32, tag="xin")
                    nc.sync.dma_start(xt[:, :, :], x[8 * g + o])
                    if o == acc_init:
                        nc.scalar.mul(acc[:, :, :], xt[:, :, :], 0.125)
                    elif o == accT_init:
                        nc.scalar.mul(accT[:, :, :], accT_slice[o](xt), 0.125)
                    elif o in accT_slice:
                        nc.vector.scalar_tensor_tensor(
                            accT[:, :, :], accT_slice[o](xt), 0.125,
                            accT[:, :, :], mult, add,
                        )
                        if last and o == 7:
                            # accT complete: fold into acc now, overlapped
                            # by the o=2/6/4 loads still in flight.
                            for h0 in (0, HH):
                                hs = slice(h0, h0 + HH)
                                nc.vector.tensor_add(
                                    acc[:, hs, :], acc[:, hs, :],
                                    accT[:, :, hs].transpose([0, 2, 1]),
                                )
                    elif last and o == 4:
                        # Final tile of the whole kernel: H-halves, each
                        # stored as soon as its STT lands (rows complete —
                        # o=2/6 STTs and the combine precede on DVE).
                        for h0 in (0, HH):
                            hs = slice(h0, h0 + HH)
                            nc.vector.scalar_tensor_tensor(
                                acc[:, hs, :], xt[:, hs, ::-1], 0.125,
                                acc[:, hs, :], mult, add,
                            )
                            nc.scalar.dma_start(y[g][:, hs, :], acc[:, hs, :])
                    else:
                        nc.vector.scalar_tensor_tensor(
                            acc[:, :, :], acc_slice[o](xt), 0.125,
                            acc[:, :, :], mult, add,
                        )
                if not last:
                    # Combine + store at group end: runs during the next
                    # group's o=0/o=5 loads, whose consumers are on ACT —
                    # DVE is naturally free there. Store on the ACT HWDGE
                    # queue so the compute-gated store can't head-of-line
                    # block loads on sync's queue.
                    for h0 in (0, HH):
                        hs = slice(h0, h0 + HH)
                        nc.vector.tensor_add(
                            acc[:, hs, :], acc[:, hs, :],
                            accT[:, :, hs].transpose([0, 2, 1]),
                        )
                        nc.scalar.dma_start(y[g][:, hs, :], acc[:, hs, :])
    nc.compile()
    return nc


_NC_CACHE: list = []


def run(x: np.ndarray, trace: bool = False, **spmd_kwargs):
    """Shard, run on all 8 cores, gather. Returns (output, BassKernelResults)."""
    x = np.ascontiguousarray(x, dtype=np.float32)
    assert x.shape == (B, C, H, W), x.shape
    shards = x.reshape(N_CORES, ENTRIES_PER_CORE, C, H, W)
    if not _NC_CACHE:
        _NC_CACHE.append(build_nc())
    nc = _NC_CACHE[0]
    in_maps = [{"x": shards[i]} for i in range(N_CORES)]
    res = run_bass_kernel_spmd(
        nc, in_maps, list(range(N_CORES)), trace=trace, **spmd_kwargs
    )
    out = np.concatenate([res.results[i]["y"] for i in range(N_CORES)], axis=0)
    return out, res


def kernel(x: np.ndarray) -> np.ndarray:
    out, _ = run(x)
    return out


# revision 12
# speedup vs baseline: 1.2227x; 1.0698x over previous
"""D4-pool Trainium2 kernel.

x: [256, 128, 64, 64] f32. Groups of 8 consecutive batch entries hold the 8
D4 orientations of one image; undo each orientation and mean over the group,
giving [32, 128, 64, 64].

Sharding: data-parallel over the group dim — core k gets groups [4k, 4k+4)
(batch entries [32k, 32k+32)), so the reduce is fully device-local.

Layout trick: with C (=128) on SBUF partitions and (H, W) on the free dim,
every D4 inverse transform is pure free-dim address arithmetic (stride +-1 /
+-64 access patterns) — no transpose instructions, no partition movement.
Per partition, the required inverse-transform reads are:
  o=0: A[h, w]          o=1: A[w, 63-h]     o=2: A[63-h, 63-w]
  o=3: A[63-w, h]       o=4: A[h, 63-w]     o=5: A[w, h]
  o=6: A[63-h, w]       o=7: A[63-w, 63-h]
Loads/stores are fully contiguous 2 MiB DMAs; DVE does the accumulation
(1/8-scale folded in); ACT initializes accumulators off the critical path.
Measured ~200 us/core on hardware ~= the HBM/DMA line-rate roofline
(75.5 MB/core through 16 SDMA engines at ~27 GiB/s each).

Scheduling note: extensive experiments (phase interleaving, half-tile
streamed loads, mid-group combine, row-chunked stores) all regressed by
12-45 us despite shorter theoretical critical paths — the measured DMA
per-descriptor service time inflates when the emission pattern deviates
from this exact steady-state shape (likely cross-core HBM interference:
all 8 cores share the device at its bandwidth ceiling, and only the
uniform 8-loads-per-group cadence keeps the cores' demand smooth). Keep
the stream shape EXACTLY as below; only the fused last-group tail
(half STT + combine + store per H-half) deviates, saving ~4 us.
"""

import sys

for _p in ("/opt/trn_rl_repo",):
    if _p not in sys.path:
        sys.path.insert(0, _p)

import numpy as np

import concourse.bacc as bacc
import concourse.mybir as mybir
from concourse.bass_utils import run_bass_kernel_spmd
from concourse.tile import TileContext

N_CORES = 8
B, C, H, W = 256, 128, 64, 64
ENTRIES_PER_CORE = B // N_CORES          # 32 batch entries
GROUPS_PER_CORE = ENTRIES_PER_CORE // 8  # 4 groups of 8 orientations


def build_nc(groups: int = GROUPS_PER_CORE) -> bacc.Bacc:
    f32 = mybir.dt.float32
    nc = bacc.Bacc()
    x = nc.declare_dram_parameter("x", [groups * 8, C, H, W], f32, isOutput=False)
    y = nc.declare_dram_parameter("y", [groups, C, H, W], f32, isOutput=True)

    # Two accumulators per group so only ONE DVE op per group pays the
    # slow inner-stride-64 (transposed) read:
    #   acc  [c,h,w]: init = x0*1/8 (ACT), += o=2,4,6 (flip APs, stride +-1)
    #   accT [c,w,h]: init = x5*1/8 (ACT; pure transpose == contiguous),
    #                 += o=1,3,7 (flips in transposed coords, stride +-1)
    # The 1/8 scale folds into every accumulate (DVE STT: term*s + acc),
    # so nothing post-combine remains but the store. Combine + store run
    # in H-halves so the first half's store overlaps the second half.
    # accT-side APs: accT[w,h] += A1[w,63-h] / A3[63-w,h] / A7[63-w,63-h].
    accT_slice = {1: lambda t: t[:, :, ::-1], 3: lambda t: t[:, ::-1, :],
                  7: lambda t: t[:, ::-1, ::-1]}
    acc_slice = {2: lambda t: t[:, ::-1, ::-1], 4: lambda t: t[:, :, ::-1],
                 6: lambda t: t[:, ::-1, :]}
    mult, add = mybir.AluOpType.mult, mybir.AluOpType.add
    with TileContext(nc) as tc:
        with (
            tc.tile_pool(name="xin", bufs=8) as xin_pool,
            tc.tile_pool(name="acc", bufs=2) as acc_pool,
            tc.tile_pool(name="accT", bufs=2) as accT_pool,
        ):
            for g in range(groups):
                acc = acc_pool.tile([C, H, W], f32, tag="acc")
                accT = accT_pool.tile([C, H, W], f32, tag="accT")
                last = g == groups - 1
                for o in (0, 5, 1, 2, 3, 4, 6, 7):
                    xt = xin_pool.tile([C, H, W], f32, tag="xin")
                    nc.sync.dma_start(xt[:, :, :], x[8 * g + o])
                    if o == 0:
                        nc.scalar.mul(acc[:, :, :], xt[:, :, :], 0.125)
                    elif o == 5:
                        nc.scalar.mul(accT[:, :, :], xt[:, :, :], 0.125)
                    elif o == 7 and last:
                        # Tail of the whole kernel: process the final
                        # orientation, combine, and store in H-halves so
                        # the first half's store overlaps the second
                        # half's compute.
                        for h0 in (0, H // 2):
                            hs = slice(h0, h0 + H // 2)
                            nc.vector.scalar_tensor_tensor(
                                accT[:, :, hs], accT_slice[7](xt)[:, :, hs],
                                0.125, accT[:, :, hs], mult, add,
                            )
                            nc.vector.tensor_add(
                                acc[:, hs, :], acc[:, hs, :],
                                accT[:, :, hs].transpose([0, 2, 1]),
                            )
                            nc.scalar.dma_start(y[g][:, hs, :], acc[:, hs, :])
                    elif o in accT_slice:
                        nc.vector.scalar_tensor_tensor(
                            accT[:, :, :], accT_slice[o](xt), 0.125,
                            accT[:, :, :], mult, add,
                        )
                    else:
                        nc.vector.scalar_tensor_tensor(
                            acc[:, :, :], acc_slice[o](xt), 0.125,
                            acc[:, :, :], mult, add,
                        )
                if not last:
                    for h0 in (0, H // 2):
                        hs = slice(h0, h0 + H // 2)
                        nc.vector.tensor_add(
                            acc[:, hs, :], acc[:, hs, :],
                            accT[:, :, hs].transpose([0, 2, 1]),
                        )
                        # Store on the ACT HWDGE queue — keeps the
                        # compute-gated store from head-of-line blocking
                        # loads on sync's queue.
                        nc.scalar.dma_start(y[g][:, hs, :], acc[:, hs, :])
    nc.compile()
    return nc


_NC_CACHE: list = []


def run(x: np.ndarray, trace: bool = False, **spmd_kwargs):
    """Shard, run on all 8 cores, gather. Returns (output, BassKernelResults)."""
    x = np.ascontiguousarray(x, dtype=np.float32)
    assert x.shape == (B, C, H, W), x.shape
    shards = x.reshape(N_CORES, ENTRIES_PER_CORE, C, H, W)
    if not _NC_CACHE:
        _NC_CACHE.append(build_nc())
    nc = _NC_CACHE[0]
    in_maps = [{"x": shards[i]} for i in range(N_CORES)]
    res = run_bass_kernel_spmd(
        nc, in_maps, list(range(N_CORES)), trace=trace, **spmd_kwargs
    )
    out = np.concatenate([res.results[i]["y"] for i in range(N_CORES)], axis=0)
    return out, res


def kernel(x: np.ndarray) -> np.ndarray:
    out, _ = run(x)
    return out
